# revision 1
# baseline (speedup 1.0000x reference)
"""GatedGraphAttConvEncoder kernel for 8 Trainium2 NeuronCores.

Strategy (per sharding_hint): shard the node dim N across the 8 cores for
the (B,N,N,D) edge tensor; GSPMD inserts the all-gather of node features
needed before attention / gated aggregation. Falls back to single-device
execution if multi-device sharding is unavailable.
"""
import numpy as np
import jax
import jax.numpy as jnp
from functools import partial

B, N, D, H, FF, L = 2, 512, 128, 8, 512, 2
DK = D // H
EPS = 1e-5
N_CORES = 8


def _bn(x, g, b):
    ax = tuple(range(x.ndim - 1))
    m = x.mean(ax, keepdims=True)
    v = x.var(ax, keepdims=True)
    return (x - m) * jax.lax.rsqrt(v + EPS) * g + b


def _mha(h, p, mask):
    q = jnp.einsum('bnd,hdk->bhnk', h, p['Wq'])
    k = jnp.einsum('bnd,hdk->bhnk', h, p['Wk'])
    v = jnp.einsum('bnd,hdk->bhnk', h, p['Wv'])
    s = jnp.einsum('bhqk,bhjk->bhqj', q, k) / jnp.sqrt(jnp.float32(DK))
    s = jnp.where(mask[:, None, :, :], jnp.float32(-1e9), s)
    a = jax.nn.softmax(s, axis=-1)
    o = jnp.einsum('bhqj,bhjk->bhqk', a, v)
    return jnp.einsum('bhnk,hkd->bnd', o, p['Wo'])


def _gated_gcn(h, e, mask, p):
    Uh = h @ p['W_U'] + p['b_U']
    Vh = h @ p['W_V'] + p['b_V']
    Ah = h @ p['W_A'] + p['b_A']
    Bh = h @ p['W_B'] + p['b_B']
    Ce = e @ p['W_C'] + p['b_C']
    e_pre = Ah[:, :, None, :] + Bh[:, None, :, :] + Ce
    gates = jax.nn.sigmoid(e_pre)
    valid = (~mask)[..., None].astype(h.dtype)
    agg = jnp.einsum('bijd,bjd->bid', gates * valid, Vh)
    h_out = jax.nn.gelu(_bn(Uh + agg, p['g_h'], p['b_h']))
    e_out = jax.nn.gelu(_bn(e_pre, p['g_e'], p['b_e']))
    return h_out, e_out


def _forward(x, dist, edges, params):
    dm = dist.mean()
    dv = dist.var()
    dn = (dist - dm) * jax.lax.rsqrt(dv + EPS) * params['dist_g'] + params['dist_b']
    e = dn[..., None] * params['We'] + params['be']
    h = x
    for p in params['layers']:
        h = _bn(h + _mha(h, p, edges), p['g1'], p['b1'])
        hg, eg = _gated_gcn(h, e, edges, p)
        h = _bn(h + hg, p['g2'], p['b2'])
        e = e + eg
        ff = jax.nn.gelu(h @ p['W_f1'] + p['b_f1']) @ p['W_f2'] + p['b_f2']
        h = _bn(h + ff, p['g3'], p['b3'])
    return h


_jit_cache = {}


def _get_fn():
    if 'fn' in _jit_cache:
        return _jit_cache['fn']
    devs = jax.devices()
    fn = None
    if len(devs) >= N_CORES:
        try:
            from jax.sharding import Mesh, NamedSharding, PartitionSpec as P
            mesh = Mesh(np.array(devs[:N_CORES]), ('i',))
            s_n = NamedSharding(mesh, P(None, 'i'))      # shard axis 1 (N)
            s_rep = NamedSharding(mesh, P())
            in_sh = (s_n, s_n, s_n,
                     jax.tree_util.tree_map(lambda _: s_rep,
                                            _jit_cache['params_tree']))
            fn = jax.jit(_forward, in_shardings=in_sh,
                         out_shardings=s_n)
        except Exception:
            fn = None
    if fn is None:
        fn = jax.jit(_forward)
    _jit_cache['fn'] = fn
    return fn


def kernel(x, dist, edges, params):
    params = jax.tree_util.tree_map(jnp.asarray, params)
    _jit_cache['params_tree'] = params
    x = jnp.asarray(x, jnp.float32)
    dist = jnp.asarray(dist, jnp.float32)
    edges = jnp.asarray(edges)
    try:
        fn = _get_fn()
        out = fn(x, dist, edges, params)
        out = np.asarray(jax.device_get(out))
    except Exception:
        # last-resort: plain jit on default device
        out = np.asarray(jax.device_get(jax.jit(_forward)(x, dist, edges, params)))
    return out.astype(np.float32)
